# revision 1
# baseline (speedup 1.0000x reference)
"""Trainium2 Bass kernel for nn_BasicTransformerBlock (ragged self-attn + cross-attn + GEGLU).

Strategy: batch-parallel (sample b -> core b), everything in transposed xT [D, NP]
layout so all matmuls use natural weight layouts. Scores computed transposed
(S^T[k,q]) so softmax needs no max/transposes: exp fused with mask+scale via the
ACT bias path, denominator via an appended ones-column in V. LN gamma/beta folded
into the following projection weights host-side. f32r (full-rate fp32) matmuls.
"""
import numpy as np

B, N, D, S, CD = 8, 2048, 512, 256, 768
H, DH, INNER = 8, 64, 512
FF = 2048          # GEGLU inner
FF2 = 4096         # 2*FF
EPS = 1e-5
NCORES = 8

_CACHE = {}
LAST_RUN = {}


def _windows(NP):
    return [(o, min(512, NP - o)) for o in range(0, NP, 512)]


def _build(NP, dbg=False):
    import concourse.bass as bass
    import concourse.tile as tile
    from concourse import bacc, mybir
    from concourse.masks import make_identity

    f32r = mybir.dt.float32r
    f32 = mybir.dt.float32
    AF = mybir.ActivationFunctionType
    ALU = mybir.AluOpType
    KC = NP // 128
    WINS = _windows(NP)

    nc = bacc.Bacc("TRN2", target_bir_lowering=False, debug=False,
                   enable_asserts=True, num_devices=NCORES)
    dr = {}

    def din(name, shape):
        dr[name] = nc.dram_tensor(name, list(shape), f32, kind="ExternalInput").ap()
        return dr[name]

    xT_d = din("xT", [D, NP])
    ctxT_d = din("ctxT", [CD, S])
    kmask_d = din("kmask", [128, KC])
    vmask_d = din("vmask", [128, KC])
    for nm, shp in [("wq1", [D, INNER]), ("wk1", [D, INNER]), ("wv1", [D, INNER]),
                    ("wo1", [INNER, D]), ("wq2", [D, INNER]), ("wk2", [CD, INNER]),
                    ("wv2", [CD, INNER]), ("wo2", [INNER, D]),
                    ("wfi", [D, FF2]), ("wfo", [FF, D])]:
        din(nm, shp)
    for nm in ["qb1", "kb1", "qb2", "bo1", "bo2", "bffo"]:
        din(nm, [128, 4])
    din("vb1", [1, INNER])
    din("ffb", [128, 32])
    out_d = nc.dram_tensor("out", [NP, D], f32, kind="ExternalOutput").ap()
    dbg_d = {}
    if dbg:
        for nm, shp in [("z1d", [128, 4, NP]), ("qtd", [128, 2, NP]), ("ktd", [128, 2, NP]),
                        ("vtd", [128, KC, 260]), ("otd", [128, 4, NP]), ("x1d", [128, 4, NP]),
                        ("o2d", [128, 4, NP]), ("x2d", [128, 4, NP])]:
            dbg_d[nm] = nc.dram_tensor(nm, shp, f32, kind="ExternalOutput").ap()

    def wview(name):
        # dram [K, M] -> [128, K//128, M] chunked f32r view
        return dr[name].rearrange("(c p) m -> p c m", p=128).bitcast(f32r)

    with tile.TileContext(nc) as tc:
        with tc.tile_pool(name="G", bufs=1) as G, \
             tc.tile_pool(name="GS", bufs=1) as GS, \
             tc.tile_pool(name="GR", bufs=2) as GR, \
             tc.tile_pool(name="PSMM", bufs=2, space="PSUM") as PS, \
             tc.tile_pool(name="ZDR", bufs=1, space="DRAM") as ZDR:

            xt = G.tile([128, 4, NP], f32, tag="xt")
            xT_v = xT_d.rearrange("(c p) n -> p c n", p=128)
            for (o, w) in WINS:
                nc.sync.dma_start(out=xt[:, :, o:o + w], in_=xT_v[:, :, o:o + w])
            kmask = G.tile([128, KC], f32, tag="kmask")
            nc.sync.dma_start(out=kmask, in_=kmask_d)
            vmask = G.tile([128, KC], f32, tag="vmask")
            nc.sync.dma_start(out=vmask, in_=vmask_d)
            bias_t = {}
            for nm in ["qb1", "kb1", "qb2", "bo1", "bo2", "bffo"]:
                bias_t[nm] = G.tile([128, 4], f32, tag=nm, name=nm)
                nc.sync.dma_start(out=bias_t[nm], in_=dr[nm])
            ffbt = G.tile([128, 32], f32, tag="ffb")
            nc.sync.dma_start(out=ffbt, in_=dr["ffb"])
            vb_row = G.tile([1, INNER], f32, tag="vbrow")
            nc.sync.dma_start(out=vb_row, in_=dr["vb1"])
            vbb = G.tile([128, INNER], f32, tag="vbb")
            nc.gpsimd.partition_broadcast(vbb, vb_row)
            ones = G.tile([128, 1], f32r, tag="ones")
            nc.vector.memset(ones.bitcast(f32), 1.0)
            epst = G.tile([1, 1], f32, tag="eps")
            nc.vector.memset(epst, EPS)
            zb128 = G.tile([128, 1], f32, tag="zb")
            nc.vector.memset(zb128, 0.0)
            ident = G.tile([128, 128], f32, tag="ident")
            make_identity(nc, ident)

            def layernorm(o, w, zdst, stpool=None, stag="mm"):
                """zdst[:, :, :w] (f32r) = LN_nogb(xt[:, :, o:o+w]) over D (partition chunks).

                stpool: psum pool for the stats accumulators — kept separate
                from the projection psum slots so adjacent LN->QKV window
                chains pipeline instead of serializing on the 2 PS slots."""
                xwin = xt[:, :, o:o + w]
                xr = GS.tile([128, 4, 512], f32r, tag="xr")
                nc.vector.tensor_copy(xr[:, :, :w], xwin)
                xq = GS.tile([128, 4, 512], f32r, tag="xq")
                nc.vector.tensor_mul(xq[:, :, :w], xwin, xwin)
                pool = stpool if stpool is not None else PS
                sums = pool.tile([1, 512], f32, tag=stag, name="sums")
                sq = pool.tile([1, 512], f32, tag=stag, name="sq")
                for c in range(4):
                    nc.tensor.matmul(sums[:, :w], ones, xr[:, c, :w],
                                     start=(c == 0), stop=(c == 3))
                for c in range(4):
                    nc.tensor.matmul(sq[:, :w], ones, xq[:, c, :w],
                                     start=(c == 0), stop=(c == 3))
                rows = GR.tile([128, 512], f32, tag="strows")
                rows2 = GR.tile([128, 512], f32, tag="strows2")
                rows3 = GR.tile([128, 512], f32, tag="strows3")
                mean, msq, var, sd = (rows[i:i + 1, :w] for i in (0, 32, 64, 96))
                rstd = rows2[0:1, :w]
                nmr = rows3[0:1, :w]
                nc.vector.tensor_scalar_mul(mean, sums[:, :w], 1.0 / D)
                nc.vector.tensor_mul(msq, mean, mean)
                nc.vector.scalar_tensor_tensor(out=var, in0=sq[:, :w], scalar=1.0 / D,
                                               in1=msq, op0=ALU.mult, op1=ALU.subtract)
                nc.scalar.activation(sd, var, AF.Sqrt, bias=epst)
                nc.vector.reciprocal(rstd, sd)
                nc.vector.scalar_tensor_tensor(out=nmr, in0=mean, scalar=-1.0,
                                               in1=rstd, op0=ALU.mult, op1=ALU.mult)
                rb = GR.tile([128, 512], f32, tag="rb")
                nb = GR.tile([128, 512], f32, tag="nb")
                nc.gpsimd.partition_broadcast(rb[:, :w], rstd)
                nc.gpsimd.partition_broadcast(nb[:, :w], nmr)
                tmp = GS.tile([128, 4, 512], f32r, tag="xq")
                rb_b = rb[:, :w].unsqueeze(1).broadcast_to([128, 4, w])
                nb_b = nb[:, :w].unsqueeze(1).broadcast_to([128, 4, w])
                nc.vector.tensor_mul(tmp[:, :, :w], xwin, rb_b)
                nc.vector.tensor_add(zdst[:, :, :w], tmp[:, :, :w], nb_b)

            with tc.tile_pool(name="PA", bufs=1) as PA, \
                 tc.tile_pool(name="WT", bufs=2) as WT, \
                 tc.tile_pool(name="ZW", bufs=2) as ZW, \
                 tc.tile_pool(name="ET", bufs=3) as ETP, \
                 tc.tile_pool(name="RC", bufs=2) as RCP, \
                 tc.tile_pool(name="PSC", bufs=2, space="PSUM") as SC, \
                 tc.tile_pool(name="PAV", bufs=2, space="PSUM") as PAV:

                z1dr = ZDR.tile([128, 4, NP], f32r, tag="z1dr")
                z2dr = ZDR.tile([128, 4, NP], f32r, tag="z2dr")

                def get_z(o, w, zdr, compute):
                    """Window of LN output: compute+stage to DRAM, or reload."""
                    z = ZW.tile([128, 4, 512], f32r, tag="zw", name="zw")
                    if compute:
                        layernorm(o, w, z, stpool=SC, stag="sc")
                        nc.sync.dma_start(out=zdr[:, :, o:o + w], in_=z[:, :, :w])
                    else:
                        nc.sync.dma_start(out=z[:, :, :w], in_=zdr[:, :, o:o + w])
                    return z

                ot = PA.tile([128, 4, NP], f32r, tag="ot")

                # cross-attn K2/V2 depend only on inputs: compute at the
                # very start to fill the LN pipeline-fill bubble on PE
                ctx_sb = PA.tile([128, 6, S], f32r, tag="ctx")
                nc.sync.dma_start(out=ctx_sb, in_=ctxT_d.rearrange("(c p) n -> p c n", p=128).bitcast(f32r))
                k2t = PA.tile([128, 4, S], f32r, tag="k2t")
                wk2a = WT.tile([128, 4, 512], f32r, tag="w")
                nc.sync.dma_start(out=wk2a[:, 0:3, :], in_=wview("wk2")[:, 0:3, :])
                wk2b = WT.tile([128, 4, 512], f32r, tag="w")
                nc.sync.dma_start(out=wk2b[:, 0:3, :], in_=wview("wk2")[:, 3:6, :])
                for mt in range(4):
                    kp = PS.tile([128, 512], f32, tag="mm")
                    for c in range(6):
                        wt_ = wk2a if c < 3 else wk2b
                        nc.tensor.matmul(kp[:, :S], wt_[:, c % 3, mt * 128:(mt + 1) * 128],
                                         ctx_sb[:, c, :], start=(c == 0), stop=(c == 5))
                    nc.vector.tensor_copy(k2t[:, mt, :], kp[:, :S])
                v2 = PA.tile([128, 2, 520], f32r, tag="v2")
                v2r = v2.rearrange("p k (h e) -> p k h e", e=65)
                nc.vector.memset(v2r[:, :, :, 64:65].bitcast(f32), 1.0)
                wv2a = WT.tile([128, 4, 512], f32r, tag="w")
                nc.sync.dma_start(out=wv2a[:, 0:3, :], in_=wview("wv2")[:, 0:3, :])
                wv2b = WT.tile([128, 4, 512], f32r, tag="w")
                nc.sync.dma_start(out=wv2b[:, 0:3, :], in_=wview("wv2")[:, 3:6, :])
                for tt in range(2):
                    vp2 = PS.tile([128, 512], f32, tag="mm")
                    for c in range(6):
                        wt_ = wv2a if c < 3 else wv2b
                        nc.tensor.matmul(vp2, ctx_sb[:, c, tt * 128:(tt + 1) * 128],
                                         wt_[:, c % 3, :], start=(c == 0), stop=(c == 5))
                    nc.vector.tensor_copy(v2r[:, tt, :, 0:64],
                                          vp2.rearrange("p (h e) -> p h e", e=64))


                def attention(hg, qt, kt, vt, kchunks, odst, scale,
                              kv_global=False):
                    # ragged masking lives in V (zeroed rows + 0/1 denominator
                    # column), so exp is biasless and adjacent k-chunks pair
                    # into one [128,1024] psum with a single exp.
                    for (qo, qw) in WINS:
                        for hl in range(4):
                            av = PAV.tile([65, 512], f32, tag="av")
                            mt = hl // 2
                            po = (hl % 2) * 64
                            mtk = (hg * 4 + hl) // 2 if kv_global else mt
                            vh = hg * 4 + hl if kv_global else hl
                            for kcp in range((kchunks + 1) // 2):
                                kcs = (kcp * 2, kcp * 2 + 1) if kcp * 2 + 1 < kchunks else (kcp * 2,)
                                sp = SC.tile([128, 1024], f32, tag="sc")
                                for j, kc in enumerate(kcs):
                                    nc.tensor.matmul(
                                        sp[:, j * 512:j * 512 + qw],
                                        kt[po:po + 64, mtk, kc * 128:(kc + 1) * 128],
                                        qt[po:po + 64, mt, qo:qo + qw],
                                        start=True, stop=True)
                                et = ETP.tile([128, 1024], f32r, tag="et")
                                spv = sp.rearrange("p (j q) -> p j q", j=2)
                                etv = et.rearrange("p (j q) -> p j q", j=2)
                                nj = len(kcs)
                                nc.scalar.activation(etv[:, :nj, :qw], spv[:, :nj, :qw],
                                                     AF.Exp, scale=scale, bias=zb128)
                                for j, kc in enumerate(kcs):
                                    nc.tensor.matmul(av[:, :qw],
                                                     vt[:, kc, vh * 65:(vh + 1) * 65],
                                                     et[:, j * 512:j * 512 + qw],
                                                     start=(kc == 0), stop=(kc == kchunks - 1))
                            rc = RCP.tile([1, 512], f32, tag="rc")
                            nc.vector.reciprocal(rc[:, :qw], av[64:65, :qw])
                            rcb = RCP.tile([64, 512], f32, tag="rcb")
                            nc.gpsimd.partition_broadcast(rcb[:, :qw], rc[:, :qw])
                            h = hg * 4 + hl
                            mtg = h // 2
                            pog = (h % 2) * 64
                            nc.vector.tensor_mul(odst[pog:pog + 64, mtg, qo:qo + qw],
                                                 av[0:64, :qw], rcb[:, :qw])

                # ---- attn1: per 4-head group ----
                for hg in range(2):
                    qt = PA.tile([128, 2, NP], f32r, tag="qt")
                    kt = PA.tile([128, 2, NP], f32r, tag="kt")
                    wq = WT.tile([128, 4, 512], f32r, tag="w")
                    nc.sync.dma_start(out=wq, in_=wview("wq1"))
                    wk = WT.tile([128, 4, 512], f32r, tag="w")
                    nc.sync.dma_start(out=wk, in_=wview("wk1"))
                    for (o, w) in WINS:
                        z = get_z(o, w, z1dr, compute=(hg == 0))
                        if dbg and hg == 0:
                            nc.sync.dma_start(out=dbg_d["z1d"][:, :, o:o + w],
                                              in_=z[:, :, :w].bitcast(f32))
                        for mt in range(2):
                            gmt = 2 * hg + mt
                            for dst, wsb, bt in ((qt, wq, "qb1"), (kt, wk, "kb1")):
                                pp = PS.tile([128, 512], f32, tag="mm")
                                for c in range(4):
                                    nc.tensor.matmul(pp[:, :w],
                                                     wsb[:, c, gmt * 128:(gmt + 1) * 128],
                                                     z[:, c, :w],
                                                     start=(c == 0), stop=(c == 3))
                                nc.vector.tensor_scalar_add(dst[:, mt, o:o + w], pp[:, :w],
                                                            bias_t[bt][:, gmt:gmt + 1])
                    vt = PA.tile([128, KC, 260], f32r, tag="vt")
                    vt4 = vt.rearrange("p k (h e) -> p k h e", e=65)
                    wv = WT.tile([128, 4, 512], f32r, tag="w")
                    nc.sync.dma_start(out=wv, in_=wview("wv1"))
                    for (o, w) in WINS:
                        z = get_z(o, w, z1dr, compute=False)
                        for tt in range(w // 128):
                            gtt = o // 128 + tt
                            vp = PS.tile([128, 512], f32, tag="mm")
                            for c in range(4):
                                nc.tensor.matmul(vp[:, :256],
                                                 z[:, c, tt * 128:(tt + 1) * 128],
                                                 wv[:, c, hg * 256:(hg + 1) * 256],
                                                 start=(c == 0), stop=(c == 3))
                            # masked tokens get all-zero V rows and a zero
                            # denominator-column entry, so they vanish from the
                            # softmax numerator AND sum (exp needs no mask bias;
                            # vb1==0 here, nonzero beta would need add-then-mask)
                            nc.vector.tensor_add(
                                vt4[:, gtt, :, 0:64],
                                vp[:, :256].rearrange("p (h e) -> p h e", e=64),
                                vbb[:, hg * 256:(hg + 1) * 256].rearrange("p (h e) -> p h e", e=64))
                            nc.vector.tensor_scalar_mul(
                                vt4[:, gtt, :, 0:64], vt4[:, gtt, :, 0:64],
                                vmask[:, gtt:gtt + 1])
                            nc.vector.tensor_copy(
                                vt4[:, gtt, :, 64:65],
                                vmask[:, gtt:gtt + 1].unsqueeze(1).broadcast_to([128, 4, 1]))
                    if dbg and hg == 0:
                        nc.sync.dma_start(out=dbg_d["qtd"], in_=qt.bitcast(f32))
                        nc.sync.dma_start(out=dbg_d["ktd"], in_=kt.bitcast(f32))
                        nc.sync.dma_start(out=dbg_d["vtd"], in_=vt.bitcast(f32))
                    attention(hg, qt, kt, vt, KC, ot, DH ** -0.5)

                if dbg:
                    nc.sync.dma_start(out=dbg_d["otd"], in_=ot.bitcast(f32))

                # ---- to_out1 + residual ----
                wo = WT.tile([128, 4, 512], f32r, tag="w")
                nc.sync.dma_start(out=wo, in_=wview("wo1"))
                for (o, w) in WINS:
                    for mt in range(4):
                        ap_ = PS.tile([128, 512], f32, tag="mm")
                        for c in range(4):
                            nc.tensor.matmul(ap_[:, :w], wo[:, c, mt * 128:(mt + 1) * 128],
                                             ot[:, c, o:o + w], start=(c == 0), stop=(c == 3))
                        nc.vector.scalar_tensor_tensor(
                            out=xt[:, mt, o:o + w], in0=ap_[:, :w],
                            scalar=bias_t["bo1"][:, mt:mt + 1],
                            in1=xt[:, mt, o:o + w], op0=ALU.add, op1=ALU.add)

                if dbg:
                    nc.sync.dma_start(out=dbg_d["x1d"], in_=xt)

                # ---- attn2 (cross) ----
                o2t = PA.tile([128, 4, NP], f32r, tag="ot")
                for hg in range(2):
                    q2t = PA.tile([128, 2, NP], f32r, tag="qt")
                    wq2 = WT.tile([128, 4, 512], f32r, tag="w")
                    nc.sync.dma_start(out=wq2, in_=wview("wq2"))
                    for (o, w) in WINS:
                        z = get_z(o, w, z2dr, compute=(hg == 0))
                        for mt in range(2):
                            gmt = 2 * hg + mt
                            pp = PS.tile([128, 512], f32, tag="mm")
                            for c in range(4):
                                nc.tensor.matmul(pp[:, :w],
                                                 wq2[:, c, gmt * 128:(gmt + 1) * 128],
                                                 z[:, c, :w], start=(c == 0), stop=(c == 3))
                            nc.vector.tensor_scalar_add(q2t[:, mt, o:o + w], pp[:, :w],
                                                        bias_t["qb2"][:, gmt:gmt + 1])
                    attention(hg, q2t, k2t, v2, 2, o2t, DH ** -0.5,
                              kv_global=True)

                wo2 = WT.tile([128, 4, 512], f32r, tag="w")
                nc.sync.dma_start(out=wo2, in_=wview("wo2"))
                for (o, w) in WINS:
                    for mt in range(4):
                        ap_ = PS.tile([128, 512], f32, tag="mm")
                        for c in range(4):
                            nc.tensor.matmul(ap_[:, :w], wo2[:, c, mt * 128:(mt + 1) * 128],
                                             o2t[:, c, o:o + w], start=(c == 0), stop=(c == 3))
                        nc.vector.scalar_tensor_tensor(
                            out=xt[:, mt, o:o + w], in0=ap_[:, :w],
                            scalar=bias_t["bo2"][:, mt:mt + 1],
                            in1=xt[:, mt, o:o + w], op0=ALU.add, op1=ALU.add)

                if dbg:
                    nc.sync.dma_start(out=dbg_d["o2d"], in_=o2t.bitcast(f32))
            if dbg:
                nc.sync.dma_start(out=dbg_d["x2d"], in_=xt)

            # ---- GEGLU FF ----
            with tc.tile_pool(name="PF", bufs=1) as PF, \
                 tc.tile_pool(name="GG", bufs=2) as GGP, \
                 tc.tile_pool(name="PFF", bufs=4, space="PSUM") as FFP:
                # interleave piecewise so the first-needed weight blocks land first
                wfa = PF.tile([128, 4, 2048], f32r, tag="wfa")
                wfb = PF.tile([128, 4, 2048], f32r, tag="wfb")
                wfi_v = wview("wfi")
                for i in range(4):
                    sl = slice(i * 512, (i + 1) * 512)
                    nc.sync.dma_start(out=wfa[:, :, sl], in_=wfi_v[:, :, sl])
                    nc.sync.dma_start(out=wfb[:, :, sl],
                                      in_=wfi_v[:, :, 2048 + i * 512:2048 + (i + 1) * 512])
                wfo_t = PF.tile([128, 16, 512], f32r, tag="wfo")
                wfo_v = wview("wfo")
                for i in range(4):
                    nc.sync.dma_start(out=wfo_t[:, i * 4:(i + 1) * 4, :],
                                      in_=wfo_v[:, i * 4:(i + 1) * 4, :])
                for (o, w) in WINS:
                    z3 = PF.tile([128, 4, 512], f32r, tag="z3")
                    layernorm(o, w, z3, stpool=FFP, stag="ff")
                    for half in range(w // 256):
                        ho = o + half * 256
                        hs = half * 256
                        gm = PF.tile([128, 16, 256], f32r, tag="gm")
                        for mt in range(16):
                            pa = FFP.tile([128, 512], f32, tag="ff", name="pa")
                            for c in range(4):
                                nc.tensor.matmul(pa[:, :256], wfa[:, c, mt * 128:(mt + 1) * 128],
                                                 z3[:, c, hs:hs + 256], start=(c == 0), stop=(c == 3))
                            pg = FFP.tile([128, 512], f32, tag="ff", name="pg")
                            for c in range(4):
                                nc.tensor.matmul(pg[:, :256], wfb[:, c, mt * 128:(mt + 1) * 128],
                                                 z3[:, c, hs:hs + 256], start=(c == 0), stop=(c == 3))
                            gg = GGP.tile([128, 256], f32, tag="gg")
                            nc.scalar.activation(gg, pg[:, :256], AF.Gelu,
                                                 bias=ffbt[:, 16 + mt:17 + mt])
                            nc.vector.scalar_tensor_tensor(
                                out=gm[:, mt, :], in0=pa[:, :256],
                                scalar=ffbt[:, mt:mt + 1], in1=gg,
                                op0=ALU.add, op1=ALU.mult)
                        for mto in range(4):
                            po_ = FFP.tile([128, 512], f32, tag="ff", name="po_")
                            for c in range(16):
                                nc.tensor.matmul(po_[:, :256], wfo_t[:, c, mto * 128:(mto + 1) * 128],
                                                 gm[:, c, :], start=(c == 0), stop=(c == 15))
                            nc.vector.scalar_tensor_tensor(
                                out=xt[:, mto, ho:ho + 256], in0=po_[:, :256],
                                scalar=bias_t["bffo"][:, mto:mto + 1],
                                in1=xt[:, mto, ho:ho + 256], op0=ALU.add, op1=ALU.add)
                    # transpose this window back + store (overlaps later FF windows)
                    for tt in range(o // 128, (o + w) // 128):
                        onat = PF.tile([128, 512], f32, tag="onat", bufs=2, name="onat")
                        for c in range(4):
                            tp = PS.tile([128, 512], f32, tag="mm")
                            nc.tensor.transpose(tp[:, :128], xt[:, c, tt * 128:(tt + 1) * 128], ident)
                            nc.vector.tensor_copy(onat[:, c * 128:(c + 1) * 128], tp[:, :128])
                        nc.sync.dma_start(out=out_d[tt * 128:(tt + 1) * 128, :], in_=onat)

    nc.compile()
    return nc


def _prep_inputs(inputs, NP):
    x = np.asarray(inputs["x"], dtype=np.float32)
    context = np.asarray(inputs["context"], dtype=np.float32)
    lengths = np.asarray(inputs["lengths"]).astype(np.int64)
    f = lambda k: np.asarray(inputs[k], dtype=np.float32)
    g1, b1 = f("g1"), f("b1")
    g2, b2 = f("g2"), f("b2")
    g3, b3 = f("g3"), f("b3")
    wq1, wk1, wv1 = f("wq1"), f("wk1"), f("wv1")
    wq2 = f("wq2")
    wfi = f("wff_in")

    def chunks4(v):  # [512] -> [128, 4]
        return np.ascontiguousarray(v.reshape(4, 128).T)

    shared = {
        "wq1": np.ascontiguousarray(g1[:, None] * wq1),
        "wk1": np.ascontiguousarray(g1[:, None] * wk1),
        "wv1": np.ascontiguousarray(g1[:, None] * wv1),
        "wo1": np.ascontiguousarray(f("wo1")),
        "wq2": np.ascontiguousarray(g2[:, None] * wq2),
        "wk2": np.ascontiguousarray(f("wk2")),
        "wv2": np.ascontiguousarray(f("wv2")),
        "wo2": np.ascontiguousarray(f("wo2")),
        "wfi": np.ascontiguousarray(g3[:, None] * wfi),
        "wfo": np.ascontiguousarray(f("wff_out")),
        "qb1": chunks4(b1 @ wq1),
        "kb1": chunks4(b1 @ wk1),
        "vb1": np.ascontiguousarray((b1 @ wv1).reshape(1, INNER)),
        "qb2": chunks4(b2 @ wq2),
        "bo1": chunks4(f("bo1")),
        "bo2": chunks4(f("bo2")),
        "bffo": chunks4(f("bff_out")),
        "ffb": np.ascontiguousarray((b3 @ wfi + f("bff_in")).reshape(32, 128).T),
    }
    KC = NP // 128
    in_maps = []
    for b in range(B):
        L = int(lengths[b])
        kidx = np.arange(NP).reshape(KC, 128).T  # [128, KC]
        m = {
            "xT": np.ascontiguousarray(x[b, :NP].T),
            "ctxT": np.ascontiguousarray(context[b].T),
            "kmask": np.where(kidx < L, 0.0, -30000.0).astype(np.float32),
            "vmask": (kidx < L).astype(np.float32),
        }
        m.update(shared)
        in_maps.append(m)
    return in_maps, lengths


def kernel(**inputs):
    from concourse.bass_utils import run_bass_kernel_spmd

    lengths = np.asarray(inputs["lengths"]).astype(np.int64)
    maxlen = int(lengths.max())
    NP = max(256, -(-maxlen // 256) * 256)

    if NP not in _CACHE:
        _CACHE[NP] = _build(NP)
    nc = _CACHE[NP]

    in_maps, lengths = _prep_inputs(inputs, NP)
    res = run_bass_kernel_spmd(nc, in_maps, core_ids=list(range(NCORES)))
    LAST_RUN["nc"] = nc
    LAST_RUN["NP"] = NP

    out = np.zeros((B, N, D), dtype=np.float32)
    for b in range(B):
        L = int(lengths[b])
        out[b, :L] = res.results[b]["out"][:L]
    return out

